# revision 14
# baseline (speedup 1.0000x reference)
import sys

sys.path.insert(0, "/opt/trn_rl_repo")

import numpy as np

# Problem constants (nn_Attention_34978213658826)
B, L, DM, NH, DH = 2, 2048, 1024, 16, 64
P = 128
LT = L // P            # 16 q/k tiles
MC = DM // P           # 8 m-chunks
G = 2                  # q-tiles per group for the z matmul
NG = LT // G           # 8 groups
HPC = 4                # heads per core
NPAIR = 2              # head pairs per core
NEG = -1.0e30
SCH = 1024             # scores psum chunk width
NQ = L // 512          # 4 projection column chunks

_CACHE = {}


def _ts(i, n):
    return slice(i * n, (i + 1) * n)


def build_bass(bias_on):
    import concourse.mybir as mybir
    import concourse.tile as tile
    from concourse import bacc

    f32 = mybir.dt.float32
    bf16 = mybir.dt.bfloat16
    AX = mybir.AxisListType
    AF = mybir.ActivationFunctionType

    nc = bacc.Bacc(None, target_bir_lowering=False)
    # x^T split hi/lo in bf16 (hi + lo ~= fp32-accurate contraction)
    xh_d = nc.dram_tensor("xh", [DM, L], bf16, kind="ExternalInput")
    xl_d = nc.dram_tensor("xl", [DM, L], bf16, kind="ExternalInput")
    wq_h = nc.dram_tensor("wqh", [NPAIR, DM + 1, P], bf16, kind="ExternalInput")
    wq_l = nc.dram_tensor("wql", [NPAIR, DM + 1, P], bf16, kind="ExternalInput")
    wk_h = nc.dram_tensor("wkh", [NPAIR, DM + 1, P], bf16, kind="ExternalInput")
    wk_l = nc.dram_tensor("wkl", [NPAIR, DM + 1, P], bf16, kind="ExternalInput")
    wv_d = nc.dram_tensor("wv", [DM + 1, HPC * DH], bf16, kind="ExternalInput")
    msk = nc.dram_tensor("mask", [P, P], bf16, kind="ExternalInput")
    idn = nc.dram_tensor("ident", [P, P], bf16, kind="ExternalInput")
    zout = nc.dram_tensor("zout", [NG, P, NPAIR, G * P], bf16, kind="ExternalOutput")
    wu_d = nc.dram_tensor("wu", [1, 1], f32, kind="ExternalOutput")

    with tile.TileContext(nc) as tc:
        with (
            tc.tile_pool(name="const", bufs=1) as const,
            tc.tile_pool(name="w", bufs=1) as wp,
            tc.tile_pool(name="qk", bufs=1) as qkp,
            tc.tile_pool(name="vz", bufs=1) as vzp,
            tc.tile_pool(name="xt", bufs=1) as xtp,
        ):
            ident = const.tile([P, P], bf16)
            mask = const.tile([P, P], bf16)
            ones = const.tile([1, 512], bf16) if bias_on else None
            junk = const.tile([P, 128], bf16)

            wqk = {}
            for nm in ("qh", "ql", "kh", "kl"):
                t = wp.tile([P, NPAIR, MC, P], bf16, name=f"w{nm}", tag=f"w{nm}")
                bb = (
                    wp.tile([1, NPAIR, P], bf16, name=f"w{nm}b", tag=f"w{nm}b")
                    if bias_on
                    else None
                )
                wqk[nm] = (t, bb)
            wqk_dram = {"qh": wq_h, "ql": wq_l, "kh": wk_h, "kl": wk_l}
            wv_t = wp.tile([P, MC, HPC * DH], bf16)
            wv_b = wp.tile([1, HPC * DH], bf16) if bias_on else None
            qTh = qkp.tile([P, NPAIR, L], bf16)
            qTl = qkp.tile([P, NPAIR, L], bf16)
            kTh = qkp.tile([P, NPAIR, L], bf16)
            kTl = qkp.tile([P, NPAIR, L], bf16)
            vv = vzp.tile([P, LT, HPC * DH], bf16)
            zst = [vzp.tile([P, NPAIR, G * P], bf16, name=f"zst{g}", tag=f"zst{g}") for g in range(NG)]
            xh = xtp.tile([P, MC, L], bf16)
            xl = xtp.tile([P, MC, L], bf16)

            with (
                tc.tile_pool(name="s_ps", bufs=3, space="PSUM") as s_ps,
                tc.tile_pool(name="zo_ps", bufs=1, space="PSUM") as zo_ps,
                tc.tile_pool(name="prow", bufs=2) as prowp,
                tc.tile_pool(name="pt", bufs=3) as ptp,
                tc.tile_pool(name="stat", bufs=6) as statp,
                tc.tile_pool(name="proj_ps", bufs=1, space="PSUM") as proj_ps,
            ):
                # ---- input DMA stream, ordered for earliest compute start:
                # mask/ident -> wv -> xh -> wq -> wk -> xl -> wo
                nc.vector.memset(junk, 1.0)
                nc.gpsimd.dma_start(ident, idn[:, :])
                nc.gpsimd.dma_start(mask, msk[:, :])
                if bias_on:
                    nc.vector.memset(ones, 1.0)
                nc.gpsimd.dma_start(wv_t, wv_d[:DM, :].rearrange("(c p) h -> p c h", p=P))
                if bias_on:
                    nc.gpsimd.dma_start(wv_b, wv_d[DM : DM + 1, :])
                for m in range(MC):
                    nc.gpsimd.dma_start(xh[:, m], xh_d[_ts(m, P), :])
                for nm in ("qh", "ql", "kh", "kl"):
                    t, bb = wqk[nm]
                    dram = wqk_dram[nm]
                    for _pr in range(NPAIR):
                        nc.gpsimd.dma_start(t[:, _pr], dram[_pr, :DM, :].rearrange("(c p) h -> p c h", p=P))
                        if bias_on:
                            nc.gpsimd.dma_start(bb[:, _pr], dram[_pr, DM : DM + 1, :])
                for m in range(MC):
                    nc.gpsimd.dma_start(xl[:, m], xl_d[_ts(m, P), :])

                # ---- PE warm-up: dummy matmuls bridge the x-load window so
                # HAM reaches K=8/8 before real compute starts.
                wup = statp.tile([1, 4], f32, tag="wup")
                wps = proj_ps.tile([P, 512], f32, name="wps", tag="pp")
                for w_ in range(150):
                    nc.tensor.matmul(
                        wps[:, :128], lhsT=junk, rhs=junk,
                        start=(w_ == 0), stop=(w_ == 149),
                    )
                nc.vector.reduce_max(wup[:1, :1], wps[:1, :P], axis=AX.X)
                nc.gpsimd.dma_start(wu_d[:, :], wup[:1, :1])

                def qk_proj_chunk(kind, pr, n):
                    """One 512-col chunk of the q or k projection for pair pr."""
                    th, bh = wqk[kind + "h"]
                    tl, bl = wqk[kind + "l"]
                    dest_h, dest_l = (qTh, qTl) if kind == "q" else (kTh, kTl)
                    scale = 0.125 if kind == "q" else 1.0
                    ps = s_ps.tile([P, 512], f32, name="pp", tag="s")
                    # pass order: (h,xh), (l,xh), (h,xl) — xl touched last so
                    # the chunk can start before xl finishes loading
                    for vi, (lw, rx) in enumerate((
                        (th, xh), (tl, xh), (th, xl),
                    )):
                        for m in range(MC):
                            nc.tensor.matmul(
                                ps, lhsT=lw[:, pr, m, :], rhs=rx[:, m, _ts(n, 512)],
                                start=(m == 0 and vi == 0),
                                stop=(m == MC - 1 and vi == 2 and not bias_on),
                            )
                    if bias_on:
                        nc.tensor.matmul(
                            ps, lhsT=bh[:, pr, :], rhs=ones[:, :512],
                            start=False, stop=False,
                        )
                        nc.tensor.matmul(
                            ps, lhsT=bl[:, pr, :], rhs=ones[:, :512],
                            start=False, stop=True,
                        )
                    nc.scalar.mul(dest_h[:, pr, _ts(n, 512)], ps, scale)
                    nc.vector.scalar_tensor_tensor(
                        dest_l[:, pr, _ts(n, 512)], ps, scale,
                        dest_h[:, pr, _ts(n, 512)],
                        op0=mybir.AluOpType.mult, op1=mybir.AluOpType.subtract,
                    )

                def v_proj(lt):
                    ps = s_ps.tile([P, HPC * DH], f32, name="vps", tag="s")
                    nbias = 1 if bias_on else 0
                    for m in range(MC):
                        nc.tensor.matmul(
                            ps, lhsT=xh[:, m, _ts(lt, P)], rhs=wv_t[:, m, :],
                            start=(m == 0), stop=(m == MC - 1 and nbias == 0),
                        )
                    if bias_on:
                        nc.tensor.matmul(
                            ps, lhsT=ones[:, :P], rhs=wv_b,
                            start=False, stop=True,
                        )
                    nc.scalar.copy(vv[:, lt, :], ps)

                ptgs = {}

                def emit_S_qtile(pr, g, s):
                    if s == 0:
                        ptgs[(pr, g)] = ptp.tile(
                            [P, G, 2, LT, P], bf16, name="ptg", tag="ptg"
                        )
                    ptg = ptgs[(pr, g)]
                    i = g * G + s
                    klen = (i + 1) * P
                    nch = (klen + SCH - 1) // SCH
                    # exact shape so the transpose source is contiguous 2D
                    prow = prowp.tile([P, 2, klen], bf16, name="prow", tag="prow")
                    sps2 = [[], []]
                    # interleave the two heads' chunk matmuls (K=64 row-tiled
                    # pairs run concurrently on the PE)
                    for c in range(nch):
                        cw = min(SCH, klen - c * SCH)
                        dlo = klen - P - c * SCH  # diag block offset
                        has_diag = 0 <= dlo < cw
                        sp2 = [s_ps.tile([P, SCH], f32, name=f"sp{h2}", tag="s") for h2 in range(2)]
                        for w0 in range(0, cw, 512):
                            ww = min(512, cw - w0)
                            last_piece = w0 + 512 >= cw
                            for vi, (lq, lk) in enumerate(
                                ((qTh, kTh), (qTl, kTh), (qTh, kTl))
                            ):
                                for h2 in range(2):
                                    nc.tensor.matmul(
                                        sp2[h2][:, w0 : w0 + ww],
                                        lhsT=lq[_ts(h2, DH), pr, _ts(i, P)],
                                        rhs=lk[_ts(h2, DH), pr, c * SCH + w0 : c * SCH + w0 + ww],
                                        start=(vi == 0),
                                        stop=(vi == 2 and not (has_diag and last_piece)),
                                    )
                        if has_diag:
                            for h2 in range(2):
                                nc.tensor.matmul(
                                    sp2[h2][:, dlo : dlo + P],
                                    lhsT=ident,
                                    rhs=mask,
                                    start=False,
                                    stop=True,
                                )
                        for h2 in range(2):
                            sps2[h2].append((sp2[h2], cw))
                    for h2 in range(2):
                        # two-level softmax: exp each chunk against its LOCAL
                        # max (frees psum fast), then fold the global rescale
                        # exp(m_c - m) and 1/sum into the per-chunk
                        # normalization scalar.
                        sps = sps2[h2]
                        negmc = statp.tile([P, 4], f32, tag="negmc")
                        sums = statp.tile([P, 4], f32, tag="sums")
                        for c, (sp, cw) in enumerate(sps):
                            nc.vector.reduce_max(
                                negmc[:, c : c + 1], sp[:, :cw], axis=AX.X, negate=True
                            )
                            nc.scalar.activation(
                                prow[:, h2, c * SCH : c * SCH + cw],
                                sp[:, :cw],
                                AF.Exp,
                                bias=negmc[:, c : c + 1],
                                accum_out=sums[:, c : c + 1],
                            )
                        sinv = statp.tile([P, 1], f32, tag="sinv")
                        if nch > 1:
                            negmg = statp.tile([P, 1], f32, tag="negmg")
                            nc.vector.tensor_reduce(
                                negmg, negmc[:, :nch], axis=AX.X, op=mybir.AluOpType.min
                            )
                            rsc = statp.tile([P, 4], f32, tag="rsc")
                            nc.scalar.activation(
                                rsc[:, :nch], negmc[:, :nch], AF.Exp,
                                bias=negmg, scale=-1.0,
                            )
                            ssc = statp.tile([P, 4], f32, tag="ssc")
                            nc.vector.tensor_mul(ssc[:, :nch], sums[:, :nch], rsc[:, :nch])
                            stot = statp.tile([P, 1], f32, tag="stot")
                            nc.vector.reduce_sum(stot, ssc[:, :nch], axis=AX.X)
                            nc.vector.reciprocal(sinv, stot)
                            wsc = statp.tile([P, 4], f32, tag="wsc")
                            nc.vector.tensor_scalar_mul(wsc[:, :nch], rsc[:, :nch], sinv)
                            for c, (sp, cw) in enumerate(sps):
                                nc.vector.tensor_scalar_mul(
                                    prow[:, h2, c * SCH : c * SCH + cw],
                                    prow[:, h2, c * SCH : c * SCH + cw],
                                    wsc[:, c : c + 1],
                                )
                                nc.sync.dma_start_transpose(
                                    ptg[:, s, h2, c * (SCH // P) : c * (SCH // P) + cw // P, :],
                                    prow[:, h2, c * SCH : c * SCH + cw],
                                )
                        else:
                            nc.vector.reciprocal(sinv, sums[:, :1])
                            nc.vector.tensor_scalar_mul(
                                prow[:, h2, :klen], prow[:, h2, :klen], sinv
                            )
                            nc.sync.dma_start_transpose(
                                ptg[:, s, h2, : i + 1, :], prow[:, h2, :]
                            )

                def emit_Z(pr, g):
                    ptg = ptgs[(pr, g)]
                    zps = zo_ps.tile([P, G * P], f32, name="zps", tag="zo")
                    jmax = G * (g + 1)
                    for j in range(jmax):
                        sc = max(0, j - G * g)
                        for h2 in range(2):
                            hcol = (pr * 2 + h2) * DH
                            # col-tiled: h2=0 -> psum partitions 0-63,
                            # h2=1 -> 64-127; the two matmuls run
                            # concurrently on different array column groups
                            nc.tensor.matmul(
                                zps[_ts(h2, DH), sc * P :],
                                lhsT=vv[:, j, hcol : hcol + DH],
                                rhs=ptg[:, sc:G, h2, j, :],
                                start=(j == 0),
                                stop=(j == jmax - 1),
                            )
                    nc.scalar.copy(zst[g][:, pr, :], zps)
                    if pr == 1:
                        nc.gpsimd.dma_start(zout[g], zst[g])

                # ---------------- schedule ----------------
                # Slot order interleaves the two head-pairs, runs the big
                # groups while projection filler still exists, and ends on
                # the tiny groups so the serial drain is short.
                order = [(0, 1), (0, 2), (1, 2), (0, 3), (1, 3), (0, 4),
                         (1, 4), (0, 5), (1, 5), (0, 6), (1, 6), (0, 7),
                         (1, 7), (1, 1), (0, 0), (1, 0)]

                # dense-PE filler per slot, scheduled to meet the S-stream's
                # data deadlines (>=2 slots of margin) while keeping late
                # slots fed.
                filler = {
                    0: [("P", "k", 1, 0), ("P", "k", 1, 1)],
                    1: [("P", "q", 1, 1), ("P", "k", 0, 2)],
                    2: [("P", "q", 0, 2), ("V", 10)],
                    3: [("P", "k", 1, 2), ("V", 11)],
                    4: [("P", "q", 1, 2), ("V", 12)],
                    5: [("P", "k", 0, 3), ("V", 13)],
                    6: [("P", "q", 0, 3), ("V", 14)],
                    7: [("P", "k", 1, 3), ("V", 15)],
                    8: [("P", "q", 1, 3), ("P", "q", 1, 0)],
                }


                def run_filler(item):
                    if item[0] == "P":
                        qk_proj_chunk(item[1], item[2], item[3])
                    elif item[0] == "V":
                        v_proj(item[1])
                    else:
                        emit_Z(item[1], item[2])

                # prefix: stripe V tiles between the pair-0 projections so
                # psum evacuations overlap other matmuls
                for lt in (0, 1, 2, 3, 4):
                    v_proj(lt)
                qk_proj_chunk("q", 0, 0)
                for lt in (5, 6, 7):
                    v_proj(lt)
                qk_proj_chunk("k", 0, 0)
                v_proj(8)
                qk_proj_chunk("k", 0, 1)
                v_proj(9)
                qk_proj_chunk("q", 0, 1)

                ZLAG = 2
                pending = []  # (due_slot, item)
                for t, (pr, g) in enumerate(order):
                    due = [it for dd, it in pending if dd <= t]
                    pending = [(dd, it) for dd, it in pending if dd > t]
                    work = due + filler.get(t, [])
                    emit_S_qtile(pr, g, 0)
                    if work:
                        run_filler(work[0])
                    emit_S_qtile(pr, g, 1)
                    for it in work[1:]:
                        run_filler(it)
                    pending.append((t + ZLAG, ("Z", pr, g)))
                for dd, it in sorted(pending):
                    run_filler(it)

    nc.finalize()
    return nc


def _split_bf16(a):
    import ml_dtypes

    hi = a.astype(ml_dtypes.bfloat16)
    lo = (a - hi.astype(np.float32)).astype(ml_dtypes.bfloat16)
    return hi, lo


def make_in_maps(normal_pre_resid, W_Q, W_K, W_V, W_O, b_Q, b_K, b_V, b_O):
    import ml_dtypes

    x = np.asarray(normal_pre_resid, np.float32)
    W_Q = np.asarray(W_Q, np.float32)
    W_K = np.asarray(W_K, np.float32)
    W_V = np.asarray(W_V, np.float32)
    W_O = np.asarray(W_O, np.float32)
    b_Q = np.asarray(b_Q, np.float32)
    b_K = np.asarray(b_K, np.float32)
    b_V = np.asarray(b_V, np.float32)

    mask = np.triu(np.full((P, P), NEG, np.float32), k=1).astype(ml_dtypes.bfloat16)
    ident = np.eye(P, dtype=np.float32).astype(ml_dtypes.bfloat16)
    in_maps = []
    for c in range(8):
        b, hg = divmod(c, 4)
        heads = [4 * hg + j for j in range(HPC)]
        xT = np.ascontiguousarray(x[b].T)  # [DM, L]
        xh, xl = _split_bf16(xT)

        def pack_qk(W, bias):
            prs = []
            for p_ in range(NPAIR):
                h0, h1 = heads[2 * p_], heads[2 * p_ + 1]
                wcat = np.concatenate([W[h0], W[h1]], axis=1)  # [DM, 128]
                bcat = np.concatenate([bias[h0], bias[h1]])[None, :]
                prs.append(np.concatenate([wcat, bcat], axis=0))  # [DM+1, 128]
            return _split_bf16(np.ascontiguousarray(np.stack(prs)))

        wqh, wql = pack_qk(W_Q, b_Q)
        wkh, wkl = pack_qk(W_K, b_K)
        wv_cat = np.concatenate([W_V[h] for h in heads], axis=1)
        bv_cat = np.concatenate([b_V[h] for h in heads])[None, :]
        wv_full = np.concatenate([wv_cat, bv_cat], axis=0).astype(ml_dtypes.bfloat16)
        in_maps.append(
            {
                "xh": np.ascontiguousarray(xh),
                "xl": np.ascontiguousarray(xl),
                "wqh": wqh,
                "wql": wql,
                "wkh": wkh,
                "wkl": wkl,
                "wv": np.ascontiguousarray(wv_full),
                "mask": mask,
                "ident": ident,
            }
        )
    return in_maps


def run_device(in_maps, bias_on=False, **kwargs):
    from concourse.bass_utils import run_bass_kernel_spmd

    key = ("nc", bias_on)
    if key not in _CACHE:
        _CACHE[key] = build_bass(bias_on)
    return run_bass_kernel_spmd(_CACHE[key], in_maps, core_ids=list(range(8)), **kwargs)


def kernel(normal_pre_resid, W_Q, W_K, W_V, W_O, b_Q, b_K, b_V, b_O, **extra):
    b_O = np.asarray(b_O, np.float32)
    bias_on = any(
        float(np.max(np.abs(np.asarray(bb, np.float32)))) > 0.0
        for bb in (b_Q, b_K, b_V)
    )
    in_maps = make_in_maps(
        normal_pre_resid, W_Q, W_K, W_V, W_O, b_Q, b_K, b_V, b_O
    )
    res = run_device(in_maps, bias_on=bias_on)
    W_O = np.asarray(W_O, np.float32)
    full = np.zeros((B, L, DM), np.float32)
    for c in range(8):
        b, hg = divmod(c, 4)
        heads = [4 * hg + j for j in range(HPC)]
        zo = np.asarray(res.results[c]["zout"], np.float32)  # [NG,P,NPAIR,G*P]
        # zo[g, h2*DH+hd, pr, s*P+qq] = z for q = g*G*P + s*P + qq,
        # head = heads[pr*2+h2], dim hd
        zo = zo.reshape(NG, 2, DH, NPAIR, G, P)
        z = zo.transpose(0, 4, 5, 3, 1, 2).reshape(L, NPAIR * 2, DH)
        for pi in range(NPAIR * 2):
            full[b] += z[:, pi, :] @ W_O[heads[pi]]
    full += b_O[None, None, :]
    return full


# revision 15
# speedup vs baseline: 1.0194x; 1.0194x over previous
import sys

sys.path.insert(0, "/opt/trn_rl_repo")

import numpy as np

# Problem constants (nn_Attention_34978213658826)
B, L, DM, NH, DH = 2, 2048, 1024, 16, 64
P = 128
LT = L // P            # 16 q/k tiles
MC = DM // P           # 8 m-chunks
G = 2                  # q-tiles per group for the z matmul
NG = LT // G           # 8 groups
HPC = 4                # heads per core
NPAIR = 2              # head pairs per core
NEG = -1.0e30
SCH = 1024             # scores psum chunk width
NQ = L // 512          # 4 projection column chunks

_CACHE = {}


def _ts(i, n):
    return slice(i * n, (i + 1) * n)


def build_bass(bias_on):
    import concourse.mybir as mybir
    import concourse.tile as tile
    from concourse import bacc

    f32 = mybir.dt.float32
    bf16 = mybir.dt.bfloat16
    AX = mybir.AxisListType
    AF = mybir.ActivationFunctionType

    nc = bacc.Bacc(None, target_bir_lowering=False)
    # x^T split hi/lo in bf16 (hi + lo ~= fp32-accurate contraction)
    xh_d = nc.dram_tensor("xh", [DM, L], bf16, kind="ExternalInput")
    xl_d = nc.dram_tensor("xl", [DM, L], bf16, kind="ExternalInput")
    wq_h = nc.dram_tensor("wqh", [NPAIR, DM + 1, P], bf16, kind="ExternalInput")
    wq_l = nc.dram_tensor("wql", [NPAIR, DM + 1, P], bf16, kind="ExternalInput")
    wk_h = nc.dram_tensor("wkh", [NPAIR, DM + 1, P], bf16, kind="ExternalInput")
    wk_l = nc.dram_tensor("wkl", [NPAIR, DM + 1, P], bf16, kind="ExternalInput")
    wv_d = nc.dram_tensor("wv", [DM + 1, HPC * DH], bf16, kind="ExternalInput")
    msk = nc.dram_tensor("mask", [P, P], bf16, kind="ExternalInput")
    idn = nc.dram_tensor("ident", [P, P], bf16, kind="ExternalInput")
    zout = nc.dram_tensor("zout", [NG, P, NPAIR, G * P], bf16, kind="ExternalOutput")
    wu_d = nc.dram_tensor("wu", [1, 1], f32, kind="ExternalOutput")

    with tile.TileContext(nc) as tc:
        with (
            tc.tile_pool(name="const", bufs=1) as const,
            tc.tile_pool(name="w", bufs=1) as wp,
            tc.tile_pool(name="qk", bufs=1) as qkp,
            tc.tile_pool(name="vz", bufs=1) as vzp,
            tc.tile_pool(name="xt", bufs=1) as xtp,
        ):
            ident = const.tile([P, P], bf16)
            mask = const.tile([P, P], bf16)
            ones = const.tile([1, 512], bf16) if bias_on else None
            junk = const.tile([P, 128], bf16)

            wqk = {}
            for nm in ("qh", "ql", "kh", "kl"):
                t = wp.tile([P, NPAIR, MC, P], bf16, name=f"w{nm}", tag=f"w{nm}")
                bb = (
                    wp.tile([1, NPAIR, P], bf16, name=f"w{nm}b", tag=f"w{nm}b")
                    if bias_on
                    else None
                )
                wqk[nm] = (t, bb)
            wqk_dram = {"qh": wq_h, "ql": wq_l, "kh": wk_h, "kl": wk_l}
            wv_t = wp.tile([P, MC, HPC * DH], bf16)
            wv_b = wp.tile([1, HPC * DH], bf16) if bias_on else None
            qTh = qkp.tile([P, NPAIR, L], bf16)
            qTl = qkp.tile([P, NPAIR, L], bf16)
            kTh = qkp.tile([P, NPAIR, L], bf16)
            kTl = qkp.tile([P, NPAIR, L], bf16)
            vv = vzp.tile([P, LT, HPC * DH], bf16)
            zst = [vzp.tile([P, NPAIR, G * P], bf16, name=f"zst{g}", tag=f"zst{g}") for g in range(NG)]
            xh = xtp.tile([P, MC, L], bf16)
            xl = xtp.tile([P, MC, L], bf16)

            with (
                tc.tile_pool(name="s_ps", bufs=3, space="PSUM") as s_ps,
                tc.tile_pool(name="zo_ps", bufs=1, space="PSUM") as zo_ps,
                tc.tile_pool(name="prow", bufs=3) as prowp,
                tc.tile_pool(name="pt", bufs=3) as ptp,
                tc.tile_pool(name="stat", bufs=6) as statp,
                tc.tile_pool(name="proj_ps", bufs=1, space="PSUM") as proj_ps,
            ):
                # ---- input DMA stream, ordered for earliest compute start:
                # mask/ident -> wv -> xh -> wq -> wk -> xl -> wo
                nc.vector.memset(junk, 1.0)
                nc.gpsimd.dma_start(ident, idn[:, :])
                nc.gpsimd.dma_start(mask, msk[:, :])
                if bias_on:
                    nc.vector.memset(ones, 1.0)
                nc.gpsimd.dma_start(wv_t, wv_d[:DM, :].rearrange("(c p) h -> p c h", p=P))
                if bias_on:
                    nc.gpsimd.dma_start(wv_b, wv_d[DM : DM + 1, :])
                for m in range(MC):
                    nc.gpsimd.dma_start(xh[:, m], xh_d[_ts(m, P), :])
                for nm in ("qh", "ql", "kh", "kl"):
                    t, bb = wqk[nm]
                    dram = wqk_dram[nm]
                    for _pr in range(NPAIR):
                        nc.gpsimd.dma_start(t[:, _pr], dram[_pr, :DM, :].rearrange("(c p) h -> p c h", p=P))
                        if bias_on:
                            nc.gpsimd.dma_start(bb[:, _pr], dram[_pr, DM : DM + 1, :])
                for m in range(MC):
                    nc.gpsimd.dma_start(xl[:, m], xl_d[_ts(m, P), :])

                # ---- PE warm-up: dummy matmuls bridge the x-load window so
                # HAM reaches K=8/8 before real compute starts.
                wup = statp.tile([1, 4], f32, tag="wup")
                wps = proj_ps.tile([P, 512], f32, name="wps", tag="pp")
                for w_ in range(150):
                    nc.tensor.matmul(
                        wps[:, :128], lhsT=junk, rhs=junk,
                        start=(w_ == 0), stop=(w_ == 149),
                    )
                nc.vector.reduce_max(wup[:1, :1], wps[:1, :P], axis=AX.X)
                nc.gpsimd.dma_start(wu_d[:, :], wup[:1, :1])

                def qk_proj_chunk(kind, pr, n):
                    """One 512-col chunk of the q or k projection for pair pr."""
                    th, bh = wqk[kind + "h"]
                    tl, bl = wqk[kind + "l"]
                    dest_h, dest_l = (qTh, qTl) if kind == "q" else (kTh, kTl)
                    scale = 0.125 if kind == "q" else 1.0
                    ps = s_ps.tile([P, 512], f32, name="pp", tag="s")
                    # pass order: (h,xh), (l,xh), (h,xl) — xl touched last so
                    # the chunk can start before xl finishes loading
                    for vi, (lw, rx) in enumerate((
                        (th, xh), (tl, xh), (th, xl),
                    )):
                        for m in range(MC):
                            nc.tensor.matmul(
                                ps, lhsT=lw[:, pr, m, :], rhs=rx[:, m, _ts(n, 512)],
                                start=(m == 0 and vi == 0),
                                stop=(m == MC - 1 and vi == 2 and not bias_on),
                            )
                    if bias_on:
                        nc.tensor.matmul(
                            ps, lhsT=bh[:, pr, :], rhs=ones[:, :512],
                            start=False, stop=False,
                        )
                        nc.tensor.matmul(
                            ps, lhsT=bl[:, pr, :], rhs=ones[:, :512],
                            start=False, stop=True,
                        )
                    nc.scalar.mul(dest_h[:, pr, _ts(n, 512)], ps, scale)
                    nc.vector.scalar_tensor_tensor(
                        dest_l[:, pr, _ts(n, 512)], ps, scale,
                        dest_h[:, pr, _ts(n, 512)],
                        op0=mybir.AluOpType.mult, op1=mybir.AluOpType.subtract,
                    )

                def v_proj(lt):
                    ps = s_ps.tile([P, HPC * DH], f32, name="vps", tag="s")
                    nbias = 1 if bias_on else 0
                    for m in range(MC):
                        nc.tensor.matmul(
                            ps, lhsT=xh[:, m, _ts(lt, P)], rhs=wv_t[:, m, :],
                            start=(m == 0), stop=(m == MC - 1 and nbias == 0),
                        )
                    if bias_on:
                        nc.tensor.matmul(
                            ps, lhsT=ones[:, :P], rhs=wv_b,
                            start=False, stop=True,
                        )
                    nc.scalar.copy(vv[:, lt, :], ps)

                ptgs = {}

                def emit_S_qtile(pr, g, s):
                    if s == 0:
                        ptgs[(pr, g)] = ptp.tile(
                            [P, G, 2, LT, P], bf16, name="ptg", tag="ptg"
                        )
                    ptg = ptgs[(pr, g)]
                    i = g * G + s
                    klen = (i + 1) * P
                    nch = (klen + SCH - 1) // SCH
                    # exact shape so the transpose source is contiguous 2D
                    prow = prowp.tile([P, 2, klen], bf16, name="prow", tag="prow")
                    sps2 = [[], []]
                    # interleave the two heads' chunk matmuls (K=64 row-tiled
                    # pairs run concurrently on the PE)
                    for c in range(nch):
                        cw = min(SCH, klen - c * SCH)
                        dlo = klen - P - c * SCH  # diag block offset
                        has_diag = 0 <= dlo < cw
                        sp2 = [s_ps.tile([P, SCH], f32, name=f"sp{h2}", tag="s") for h2 in range(2)]
                        for w0 in range(0, cw, 512):
                            ww = min(512, cw - w0)
                            last_piece = w0 + 512 >= cw
                            for vi, (lq, lk) in enumerate(
                                ((qTh, kTh), (qTl, kTh), (qTh, kTl))
                            ):
                                for h2 in range(2):
                                    nc.tensor.matmul(
                                        sp2[h2][:, w0 : w0 + ww],
                                        lhsT=lq[_ts(h2, DH), pr, _ts(i, P)],
                                        rhs=lk[_ts(h2, DH), pr, c * SCH + w0 : c * SCH + w0 + ww],
                                        start=(vi == 0),
                                        stop=(vi == 2 and not (has_diag and last_piece)),
                                    )
                        if has_diag:
                            for h2 in range(2):
                                nc.tensor.matmul(
                                    sp2[h2][:, dlo : dlo + P],
                                    lhsT=ident,
                                    rhs=mask,
                                    start=False,
                                    stop=True,
                                )
                        for h2 in range(2):
                            sps2[h2].append((sp2[h2], cw))
                    for h2 in range(2):
                        # two-level softmax: exp each chunk against its LOCAL
                        # max (frees psum fast), then fold the global rescale
                        # exp(m_c - m) and 1/sum into the per-chunk
                        # normalization scalar.
                        sps = sps2[h2]
                        negmc = statp.tile([P, 4], f32, tag="negmc")
                        sums = statp.tile([P, 4], f32, tag="sums")
                        for c, (sp, cw) in enumerate(sps):
                            nc.vector.reduce_max(
                                negmc[:, c : c + 1], sp[:, :cw], axis=AX.X, negate=True
                            )
                            nc.scalar.activation(
                                prow[:, h2, c * SCH : c * SCH + cw],
                                sp[:, :cw],
                                AF.Exp,
                                bias=negmc[:, c : c + 1],
                                accum_out=sums[:, c : c + 1],
                            )
                        sinv = statp.tile([P, 1], f32, tag="sinv")
                        if nch > 1:
                            negmg = statp.tile([P, 1], f32, tag="negmg")
                            nc.vector.tensor_reduce(
                                negmg, negmc[:, :nch], axis=AX.X, op=mybir.AluOpType.min
                            )
                            rsc = statp.tile([P, 4], f32, tag="rsc")
                            nc.scalar.activation(
                                rsc[:, :nch], negmc[:, :nch], AF.Exp,
                                bias=negmg, scale=-1.0,
                            )
                            ssc = statp.tile([P, 4], f32, tag="ssc")
                            nc.vector.tensor_mul(ssc[:, :nch], sums[:, :nch], rsc[:, :nch])
                            stot = statp.tile([P, 1], f32, tag="stot")
                            nc.vector.reduce_sum(stot, ssc[:, :nch], axis=AX.X)
                            nc.vector.reciprocal(sinv, stot)
                            wsc = statp.tile([P, 4], f32, tag="wsc")
                            nc.vector.tensor_scalar_mul(wsc[:, :nch], rsc[:, :nch], sinv)
                            for c, (sp, cw) in enumerate(sps):
                                nc.vector.tensor_scalar_mul(
                                    prow[:, h2, c * SCH : c * SCH + cw],
                                    prow[:, h2, c * SCH : c * SCH + cw],
                                    wsc[:, c : c + 1],
                                )
                        else:
                            nc.vector.reciprocal(sinv, sums[:, :1])
                            nc.vector.tensor_scalar_mul(
                                prow[:, h2, :klen], prow[:, h2, :klen], sinv
                            )
                        nc.sync.dma_start_transpose(
                            ptg[:, s, h2, : i + 1, :], prow[:, h2, :]
                        )

                def emit_Z(pr, g):
                    ptg = ptgs[(pr, g)]
                    zps = zo_ps.tile([P, G * P], f32, name="zps", tag="zo")
                    jmax = G * (g + 1)
                    for j in range(jmax):
                        sc = max(0, j - G * g)
                        for h2 in range(2):
                            hcol = (pr * 2 + h2) * DH
                            # col-tiled: h2=0 -> psum partitions 0-63,
                            # h2=1 -> 64-127; the two matmuls run
                            # concurrently on different array column groups
                            nc.tensor.matmul(
                                zps[_ts(h2, DH), sc * P :],
                                lhsT=vv[:, j, hcol : hcol + DH],
                                rhs=ptg[:, sc:G, h2, j, :],
                                start=(j == 0),
                                stop=(j == jmax - 1),
                            )
                    nc.scalar.copy(zst[g][:, pr, :], zps)
                    nc.gpsimd.dma_start(
                        zout[g, :, pr : pr + 1, :], zst[g][:, pr : pr + 1, :]
                    )

                # ---------------- schedule ----------------
                # Slot order interleaves the two head-pairs, runs the big
                # groups while projection filler still exists, and ends on
                # the tiny groups so the serial drain is short.
                order = [(0, 1), (0, 2), (1, 2), (0, 3), (1, 3), (0, 4),
                         (1, 4), (0, 5), (1, 5), (0, 6), (1, 6), (0, 7),
                         (1, 7), (1, 1), (0, 0), (1, 0)]

                # dense-PE filler per slot, scheduled to meet the S-stream's
                # data deadlines (>=2 slots of margin) while keeping late
                # slots fed.
                filler = {
                    0: [("P", "k", 1, 0), ("P", "k", 1, 1)],
                    1: [("P", "q", 1, 1), ("P", "k", 0, 2)],
                    2: [("P", "q", 0, 2), ("V", 10)],
                    3: [("P", "k", 1, 2), ("V", 11)],
                    4: [("P", "q", 1, 2), ("V", 12)],
                    5: [("P", "k", 0, 3), ("V", 13)],
                    6: [("P", "q", 0, 3), ("V", 14)],
                    7: [("P", "k", 1, 3), ("V", 15)],
                    8: [("P", "q", 1, 3), ("P", "q", 1, 0)],
                }


                def run_filler(item):
                    if item[0] == "P":
                        qk_proj_chunk(item[1], item[2], item[3])
                    elif item[0] == "V":
                        v_proj(item[1])
                    else:
                        emit_Z(item[1], item[2])

                # prefix: stripe V tiles between the pair-0 projections so
                # psum evacuations overlap other matmuls
                for lt in (0, 1, 2, 3, 4):
                    v_proj(lt)
                qk_proj_chunk("q", 0, 0)
                for lt in (5, 6, 7):
                    v_proj(lt)
                qk_proj_chunk("k", 0, 0)
                v_proj(8)
                qk_proj_chunk("k", 0, 1)
                v_proj(9)
                qk_proj_chunk("q", 0, 1)

                ZLAG = 2
                pending = []  # (due_slot, item)
                for t, (pr, g) in enumerate(order):
                    due = [it for dd, it in pending if dd <= t]
                    pending = [(dd, it) for dd, it in pending if dd > t]
                    # Z items last: their transposes get the whole slot to land
                    work = filler.get(t, []) + due
                    emit_S_qtile(pr, g, 0)
                    if work:
                        run_filler(work[0])
                    emit_S_qtile(pr, g, 1)
                    for it in work[1:]:
                        run_filler(it)
                    zlag = ZLAG if len(order) - t > 3 else 1
                    pending.append((t + zlag, ("Z", pr, g)))
                for dd, it in sorted(pending):
                    run_filler(it)

    nc.finalize()
    return nc


def _split_bf16(a):
    import ml_dtypes

    hi = a.astype(ml_dtypes.bfloat16)
    lo = (a - hi.astype(np.float32)).astype(ml_dtypes.bfloat16)
    return hi, lo


def make_in_maps(normal_pre_resid, W_Q, W_K, W_V, W_O, b_Q, b_K, b_V, b_O):
    import ml_dtypes

    x = np.asarray(normal_pre_resid, np.float32)
    W_Q = np.asarray(W_Q, np.float32)
    W_K = np.asarray(W_K, np.float32)
    W_V = np.asarray(W_V, np.float32)
    W_O = np.asarray(W_O, np.float32)
    b_Q = np.asarray(b_Q, np.float32)
    b_K = np.asarray(b_K, np.float32)
    b_V = np.asarray(b_V, np.float32)

    mask = np.triu(np.full((P, P), NEG, np.float32), k=1).astype(ml_dtypes.bfloat16)
    ident = np.eye(P, dtype=np.float32).astype(ml_dtypes.bfloat16)
    in_maps = []
    for c in range(8):
        b, hg = divmod(c, 4)
        heads = [4 * hg + j for j in range(HPC)]
        xT = np.ascontiguousarray(x[b].T)  # [DM, L]
        xh, xl = _split_bf16(xT)

        def pack_qk(W, bias):
            prs = []
            for p_ in range(NPAIR):
                h0, h1 = heads[2 * p_], heads[2 * p_ + 1]
                wcat = np.concatenate([W[h0], W[h1]], axis=1)  # [DM, 128]
                bcat = np.concatenate([bias[h0], bias[h1]])[None, :]
                prs.append(np.concatenate([wcat, bcat], axis=0))  # [DM+1, 128]
            return _split_bf16(np.ascontiguousarray(np.stack(prs)))

        wqh, wql = pack_qk(W_Q, b_Q)
        wkh, wkl = pack_qk(W_K, b_K)
        wv_cat = np.concatenate([W_V[h] for h in heads], axis=1)
        bv_cat = np.concatenate([b_V[h] for h in heads])[None, :]
        wv_full = np.concatenate([wv_cat, bv_cat], axis=0).astype(ml_dtypes.bfloat16)
        in_maps.append(
            {
                "xh": np.ascontiguousarray(xh),
                "xl": np.ascontiguousarray(xl),
                "wqh": wqh,
                "wql": wql,
                "wkh": wkh,
                "wkl": wkl,
                "wv": np.ascontiguousarray(wv_full),
                "mask": mask,
                "ident": ident,
            }
        )
    return in_maps


def run_device(in_maps, bias_on=False, **kwargs):
    from concourse.bass_utils import run_bass_kernel_spmd

    key = ("nc", bias_on)
    if key not in _CACHE:
        _CACHE[key] = build_bass(bias_on)
    return run_bass_kernel_spmd(_CACHE[key], in_maps, core_ids=list(range(8)), **kwargs)


def kernel(normal_pre_resid, W_Q, W_K, W_V, W_O, b_Q, b_K, b_V, b_O, **extra):
    b_O = np.asarray(b_O, np.float32)
    bias_on = any(
        float(np.max(np.abs(np.asarray(bb, np.float32)))) > 0.0
        for bb in (b_Q, b_K, b_V)
    )
    in_maps = make_in_maps(
        normal_pre_resid, W_Q, W_K, W_V, W_O, b_Q, b_K, b_V, b_O
    )
    res = run_device(in_maps, bias_on=bias_on)
    W_O = np.asarray(W_O, np.float32)
    full = np.zeros((B, L, DM), np.float32)
    for c in range(8):
        b, hg = divmod(c, 4)
        heads = [4 * hg + j for j in range(HPC)]
        zo = np.asarray(res.results[c]["zout"], np.float32)  # [NG,P,NPAIR,G*P]
        # zo[g, h2*DH+hd, pr, s*P+qq] = z for q = g*G*P + s*P + qq,
        # head = heads[pr*2+h2], dim hd
        zo = zo.reshape(NG, 2, DH, NPAIR, G, P)
        z = zo.transpose(0, 4, 5, 3, 1, 2).reshape(L, NPAIR * 2, DH)
        for pi in range(NPAIR * 2):
            full[b] += z[:, pi, :] @ W_O[heads[pi]]
    full += b_O[None, None, :]
    return full


# revision 17
# speedup vs baseline: 1.0369x; 1.0172x over previous
import sys

sys.path.insert(0, "/opt/trn_rl_repo")

import numpy as np

# Problem constants (nn_Attention_34978213658826)
B, L, DM, NH, DH = 2, 2048, 1024, 16, 64
P = 128
LT = L // P            # 16 q/k tiles
MC = DM // P           # 8 m-chunks
G = 2                  # q-tiles per group for the z matmul
NG = LT // G           # 8 groups
HPC = 4                # heads per core
NPAIR = 2              # head pairs per core
NEG = -1.0e30
SCH = 1024             # scores psum chunk width
NQ = L // 512          # 4 projection column chunks

_CACHE = {}


def _ts(i, n):
    return slice(i * n, (i + 1) * n)


def build_bass(bias_on):
    import concourse.mybir as mybir
    import concourse.tile as tile
    from concourse import bacc

    f32 = mybir.dt.float32
    bf16 = mybir.dt.bfloat16
    AX = mybir.AxisListType
    AF = mybir.ActivationFunctionType

    nc = bacc.Bacc(None, target_bir_lowering=False)
    # x^T split hi/lo in bf16 (hi + lo ~= fp32-accurate contraction)
    xh_d = nc.dram_tensor("xh", [DM, L], bf16, kind="ExternalInput")
    xl_d = nc.dram_tensor("xl", [DM, L], bf16, kind="ExternalInput")
    wq_h = nc.dram_tensor("wqh", [NPAIR, DM + 1, P], bf16, kind="ExternalInput")
    wq_l = nc.dram_tensor("wql", [NPAIR, DM + 1, P], bf16, kind="ExternalInput")
    wk_h = nc.dram_tensor("wkh", [NPAIR, DM + 1, P], bf16, kind="ExternalInput")
    wk_l = nc.dram_tensor("wkl", [NPAIR, DM + 1, P], bf16, kind="ExternalInput")
    wv_d = nc.dram_tensor("wv", [DM + 1, HPC * DH], bf16, kind="ExternalInput")
    msk = nc.dram_tensor("mask", [P, P], bf16, kind="ExternalInput")
    idn = nc.dram_tensor("ident", [P, P], bf16, kind="ExternalInput")
    zout = nc.dram_tensor("zout", [NG, P, NPAIR, G * P], bf16, kind="ExternalOutput")
    S_d = nc.dram_tensor("souts", [P, LT, NPAIR, 2], f32, kind="ExternalOutput")
    wu_d = nc.dram_tensor("wu", [1, 1], f32, kind="ExternalOutput")

    with tile.TileContext(nc) as tc:
        with (
            tc.tile_pool(name="const", bufs=1) as const,
            tc.tile_pool(name="w", bufs=1) as wp,
            tc.tile_pool(name="qk", bufs=1) as qkp,
            tc.tile_pool(name="vz", bufs=1) as vzp,
            tc.tile_pool(name="xt", bufs=1) as xtp,
        ):
            ident = const.tile([P, P], bf16)
            mask = const.tile([P, P], bf16)
            ones = const.tile([1, 512], bf16) if bias_on else None
            junk = const.tile([P, 128], bf16)

            wqk = {}
            for nm in ("qh", "ql", "kh", "kl"):
                t = wp.tile([P, NPAIR, MC, P], bf16, name=f"w{nm}", tag=f"w{nm}")
                bb = (
                    wp.tile([1, NPAIR, P], bf16, name=f"w{nm}b", tag=f"w{nm}b")
                    if bias_on
                    else None
                )
                wqk[nm] = (t, bb)
            wqk_dram = {"qh": wq_h, "ql": wq_l, "kh": wk_h, "kl": wk_l}
            wv_t = wp.tile([P, MC, HPC * DH], bf16)
            wv_b = wp.tile([1, HPC * DH], bf16) if bias_on else None
            qTh = qkp.tile([P, NPAIR, L], bf16)
            qTl = qkp.tile([P, NPAIR, L], bf16)
            kTh = qkp.tile([P, NPAIR, L], bf16)
            kTl = qkp.tile([P, NPAIR, L], bf16)
            vv = vzp.tile([P, LT, HPC * DH], bf16)
            Ssb = vzp.tile([P, LT, NPAIR, 2], f32)
            zst = [vzp.tile([P, NPAIR, G * P], bf16, name=f"zst{g}", tag=f"zst{g}") for g in range(NG)]
            xh = xtp.tile([P, MC, L], bf16)
            xl = xtp.tile([P, MC, L], bf16)

            with (
                tc.tile_pool(name="s_ps", bufs=3, space="PSUM") as s_ps,
                tc.tile_pool(name="zo_ps", bufs=1, space="PSUM") as zo_ps,
                tc.tile_pool(name="prow", bufs=3) as prowp,
                tc.tile_pool(name="pt", bufs=3) as ptp,
                tc.tile_pool(name="stat", bufs=6) as statp,
                tc.tile_pool(name="proj_ps", bufs=1, space="PSUM") as proj_ps,
            ):
                # ---- input DMA stream, ordered for earliest compute start:
                # mask/ident -> wv -> xh -> wq -> wk -> xl -> wo
                nc.vector.memset(junk, 1.0)
                nc.gpsimd.dma_start(ident, idn[:, :])
                nc.gpsimd.dma_start(mask, msk[:, :])
                if bias_on:
                    nc.vector.memset(ones, 1.0)
                nc.gpsimd.dma_start(wv_t, wv_d[:DM, :].rearrange("(c p) h -> p c h", p=P))
                if bias_on:
                    nc.gpsimd.dma_start(wv_b, wv_d[DM : DM + 1, :])
                for m in range(MC):
                    nc.gpsimd.dma_start(xh[:, m], xh_d[_ts(m, P), :])
                for nm in ("qh", "ql", "kh", "kl"):
                    t, bb = wqk[nm]
                    dram = wqk_dram[nm]
                    for _pr in range(NPAIR):
                        nc.gpsimd.dma_start(t[:, _pr], dram[_pr, :DM, :].rearrange("(c p) h -> p c h", p=P))
                        if bias_on:
                            nc.gpsimd.dma_start(bb[:, _pr], dram[_pr, DM : DM + 1, :])
                for m in range(MC):
                    nc.gpsimd.dma_start(xl[:, m], xl_d[_ts(m, P), :])

                # ---- PE warm-up: dummy matmuls bridge the x-load window so
                # HAM reaches K=8/8 before real compute starts.
                wup = statp.tile([1, 4], f32, tag="wup")
                wps = proj_ps.tile([P, 512], f32, name="wps", tag="pp")
                for w_ in range(150):
                    nc.tensor.matmul(
                        wps[:, :128], lhsT=junk, rhs=junk,
                        start=(w_ == 0), stop=(w_ == 149),
                    )
                nc.vector.reduce_max(wup[:1, :1], wps[:1, :P], axis=AX.X)
                nc.gpsimd.dma_start(wu_d[:, :], wup[:1, :1])

                def qk_proj_chunk(kind, pr, n):
                    """One 512-col chunk of the q or k projection for pair pr."""
                    th, bh = wqk[kind + "h"]
                    tl, bl = wqk[kind + "l"]
                    dest_h, dest_l = (qTh, qTl) if kind == "q" else (kTh, kTl)
                    scale = 0.125 if kind == "q" else 1.0
                    ps = s_ps.tile([P, 512], f32, name="pp", tag="s")
                    # pass order: (h,xh), (l,xh), (h,xl) — xl touched last so
                    # the chunk can start before xl finishes loading
                    for vi, (lw, rx) in enumerate((
                        (th, xh), (tl, xh), (th, xl),
                    )):
                        for m in range(MC):
                            nc.tensor.matmul(
                                ps, lhsT=lw[:, pr, m, :], rhs=rx[:, m, _ts(n, 512)],
                                start=(m == 0 and vi == 0),
                                stop=(m == MC - 1 and vi == 2 and not bias_on),
                            )
                    if bias_on:
                        nc.tensor.matmul(
                            ps, lhsT=bh[:, pr, :], rhs=ones[:, :512],
                            start=False, stop=False,
                        )
                        nc.tensor.matmul(
                            ps, lhsT=bl[:, pr, :], rhs=ones[:, :512],
                            start=False, stop=True,
                        )
                    nc.scalar.mul(dest_h[:, pr, _ts(n, 512)], ps, scale)
                    nc.vector.scalar_tensor_tensor(
                        dest_l[:, pr, _ts(n, 512)], ps, scale,
                        dest_h[:, pr, _ts(n, 512)],
                        op0=mybir.AluOpType.mult, op1=mybir.AluOpType.subtract,
                    )

                def v_proj(lt):
                    ps = s_ps.tile([P, HPC * DH], f32, name="vps", tag="s")
                    nbias = 1 if bias_on else 0
                    for m in range(MC):
                        nc.tensor.matmul(
                            ps, lhsT=xh[:, m, _ts(lt, P)], rhs=wv_t[:, m, :],
                            start=(m == 0), stop=(m == MC - 1 and nbias == 0),
                        )
                    if bias_on:
                        nc.tensor.matmul(
                            ps, lhsT=ones[:, :P], rhs=wv_b,
                            start=False, stop=True,
                        )
                    nc.scalar.copy(vv[:, lt, :], ps)

                ptgs = {}

                def emit_S_qtile(pr, g, s):
                    if s == 0:
                        ptgs[(pr, g)] = ptp.tile(
                            [P, G, 2, LT, P], bf16, name="ptg", tag="ptg"
                        )
                    ptg = ptgs[(pr, g)]
                    i = g * G + s
                    klen = (i + 1) * P
                    nch = (klen + SCH - 1) // SCH
                    # exact shape so the transpose source is contiguous 2D
                    prow = prowp.tile([P, 2, klen], bf16, name="prow", tag="prow")
                    sps2 = [[], []]
                    # interleave the two heads' chunk matmuls (K=64 row-tiled
                    # pairs run concurrently on the PE)
                    for c in range(nch):
                        cw = min(SCH, klen - c * SCH)
                        dlo = klen - P - c * SCH  # diag block offset
                        has_diag = 0 <= dlo < cw
                        sp2 = [s_ps.tile([P, SCH], f32, name=f"sp{h2}", tag="s") for h2 in range(2)]
                        for w0 in range(0, cw, 512):
                            ww = min(512, cw - w0)
                            last_piece = w0 + 512 >= cw
                            for vi, (lq, lk) in enumerate(
                                ((qTh, kTh), (qTl, kTh), (qTh, kTl))
                            ):
                                for h2 in range(2):
                                    nc.tensor.matmul(
                                        sp2[h2][:, w0 : w0 + ww],
                                        lhsT=lq[_ts(h2, DH), pr, _ts(i, P)],
                                        rhs=lk[_ts(h2, DH), pr, c * SCH + w0 : c * SCH + w0 + ww],
                                        start=(vi == 0),
                                        stop=(vi == 2 and not (has_diag and last_piece)),
                                    )
                        if has_diag:
                            for h2 in range(2):
                                nc.tensor.matmul(
                                    sp2[h2][:, dlo : dlo + P],
                                    lhsT=ident,
                                    rhs=mask,
                                    start=False,
                                    stop=True,
                                )
                        for h2 in range(2):
                            sps2[h2].append((sp2[h2], cw))
                    for h2 in range(2):
                        # two-level softmax, normalization deferred to host:
                        # exp each chunk against its LOCAL max (frees psum
                        # fast), rescale chunks to the global max, and ship
                        # the row sums (Ssb) out — the host divides by S.
                        sps = sps2[h2]
                        sslot = Ssb[:, i, pr, h2 : h2 + 1]
                        if nch == 1:
                            negmc = statp.tile([P, 4], f32, tag="negmc")
                            sp, cw = sps2[h2][0]
                            nc.vector.reduce_max(
                                negmc[:, 0:1], sp[:, :cw], axis=AX.X, negate=True
                            )
                            nc.scalar.activation(
                                prow[:, h2, :cw],
                                sp[:, :cw],
                                AF.Exp,
                                bias=negmc[:, 0:1],
                                accum_out=sslot,
                            )
                        else:
                            negmc = statp.tile([P, 4], f32, tag="negmc")
                            sums = statp.tile([P, 4], f32, tag="sums")
                            for c, (sp, cw) in enumerate(sps):
                                nc.vector.reduce_max(
                                    negmc[:, c : c + 1], sp[:, :cw], axis=AX.X, negate=True
                                )
                                nc.scalar.activation(
                                    prow[:, h2, c * SCH : c * SCH + cw],
                                    sp[:, :cw],
                                    AF.Exp,
                                    bias=negmc[:, c : c + 1],
                                    accum_out=sums[:, c : c + 1],
                                )
                            negmg = statp.tile([P, 1], f32, tag="negmg")
                            nc.vector.tensor_reduce(
                                negmg, negmc[:, :nch], axis=AX.X, op=mybir.AluOpType.min
                            )
                            rsc = statp.tile([P, 4], f32, tag="rsc")
                            nc.scalar.activation(
                                rsc[:, :nch], negmc[:, :nch], AF.Exp,
                                bias=negmg, scale=-1.0,
                            )
                            ssc = statp.tile([P, 4], f32, tag="ssc")
                            nc.vector.tensor_mul(ssc[:, :nch], sums[:, :nch], rsc[:, :nch])
                            nc.vector.reduce_sum(sslot, ssc[:, :nch], axis=AX.X)
                            for c, (sp, cw) in enumerate(sps):
                                nc.vector.tensor_scalar_mul(
                                    prow[:, h2, c * SCH : c * SCH + cw],
                                    prow[:, h2, c * SCH : c * SCH + cw],
                                    rsc[:, c : c + 1],
                                )
                        nc.sync.dma_start_transpose(
                            ptg[:, s, h2, : i + 1, :], prow[:, h2, :]
                        )

                def emit_Z(pr, g):
                    ptg = ptgs[(pr, g)]
                    zps = zo_ps.tile([P, G * P], f32, name="zps", tag="zo")
                    jmax = G * (g + 1)
                    for j in range(jmax):
                        sc = max(0, j - G * g)
                        for h2 in range(2):
                            hcol = (pr * 2 + h2) * DH
                            # col-tiled: h2=0 -> psum partitions 0-63,
                            # h2=1 -> 64-127; the two matmuls run
                            # concurrently on different array column groups
                            nc.tensor.matmul(
                                zps[_ts(h2, DH), sc * P :],
                                lhsT=vv[:, j, hcol : hcol + DH],
                                rhs=ptg[:, sc:G, h2, j, :],
                                start=(j == 0),
                                stop=(j == jmax - 1),
                            )
                    nc.scalar.copy(zst[g][:, pr, :], zps)
                    nc.gpsimd.dma_start(
                        zout[g, :, pr : pr + 1, :], zst[g][:, pr : pr + 1, :]
                    )

                # ---------------- schedule ----------------
                # Slot order interleaves the two head-pairs, runs the big
                # groups while projection filler still exists, and ends on
                # the tiny groups so the serial drain is short.
                order = [(0, 1), (0, 2), (1, 2), (0, 3), (1, 3), (0, 4),
                         (1, 4), (0, 5), (1, 5), (0, 6), (1, 6), (0, 7),
                         (1, 7), (1, 1), (0, 0), (1, 0)]

                # dense-PE filler per slot, scheduled to meet the S-stream's
                # data deadlines (>=2 slots of margin) while keeping late
                # slots fed.
                filler = {
                    0: [("P", "k", 1, 0), ("P", "k", 1, 1)],
                    1: [("P", "q", 1, 1), ("P", "k", 0, 2)],
                    2: [("P", "q", 0, 2), ("V", 10)],
                    3: [("P", "k", 1, 2), ("V", 11)],
                    4: [("P", "q", 1, 2), ("V", 12)],
                    5: [("P", "k", 0, 3), ("V", 13)],
                    6: [("P", "q", 0, 3), ("V", 14)],
                    7: [("P", "k", 1, 3), ("V", 15)],
                    8: [("P", "q", 1, 3), ("P", "q", 1, 0)],
                }


                def run_filler(item):
                    if item[0] == "P":
                        qk_proj_chunk(item[1], item[2], item[3])
                    elif item[0] == "V":
                        v_proj(item[1])
                    else:
                        emit_Z(item[1], item[2])

                # prefix: stripe V tiles between the pair-0 projections so
                # psum evacuations overlap other matmuls
                for lt in (0, 1, 2, 3, 4):
                    v_proj(lt)
                qk_proj_chunk("q", 0, 0)
                for lt in (5, 6, 7):
                    v_proj(lt)
                qk_proj_chunk("k", 0, 0)
                v_proj(8)
                qk_proj_chunk("k", 0, 1)
                v_proj(9)
                qk_proj_chunk("q", 0, 1)

                ZLAG = 2
                pending = []  # (due_slot, item)
                for t, (pr, g) in enumerate(order):
                    due = [it for dd, it in pending if dd <= t]
                    pending = [(dd, it) for dd, it in pending if dd > t]
                    # Z items last: their transposes get the whole slot to land
                    work = filler.get(t, []) + due
                    emit_S_qtile(pr, g, 0)
                    if work:
                        run_filler(work[0])
                    emit_S_qtile(pr, g, 1)
                    for it in work[1:]:
                        run_filler(it)
                    zlag = ZLAG if len(order) - t > 3 else 1
                    pending.append((t + zlag, ("Z", pr, g)))
                for dd, it in sorted(pending):
                    run_filler(it)
                nc.gpsimd.dma_start(S_d[:, :, :, :], Ssb)

    nc.finalize()
    return nc


def _split_bf16(a):
    import ml_dtypes

    hi = a.astype(ml_dtypes.bfloat16)
    lo = (a - hi.astype(np.float32)).astype(ml_dtypes.bfloat16)
    return hi, lo


def make_in_maps(normal_pre_resid, W_Q, W_K, W_V, W_O, b_Q, b_K, b_V, b_O):
    import ml_dtypes

    x = np.asarray(normal_pre_resid, np.float32)
    W_Q = np.asarray(W_Q, np.float32)
    W_K = np.asarray(W_K, np.float32)
    W_V = np.asarray(W_V, np.float32)
    W_O = np.asarray(W_O, np.float32)
    b_Q = np.asarray(b_Q, np.float32)
    b_K = np.asarray(b_K, np.float32)
    b_V = np.asarray(b_V, np.float32)

    mask = np.triu(np.full((P, P), NEG, np.float32), k=1).astype(ml_dtypes.bfloat16)
    ident = np.eye(P, dtype=np.float32).astype(ml_dtypes.bfloat16)
    in_maps = []
    for c in range(8):
        b, hg = divmod(c, 4)
        heads = [4 * hg + j for j in range(HPC)]
        xT = np.ascontiguousarray(x[b].T)  # [DM, L]
        xh, xl = _split_bf16(xT)

        def pack_qk(W, bias):
            prs = []
            for p_ in range(NPAIR):
                h0, h1 = heads[2 * p_], heads[2 * p_ + 1]
                wcat = np.concatenate([W[h0], W[h1]], axis=1)  # [DM, 128]
                bcat = np.concatenate([bias[h0], bias[h1]])[None, :]
                prs.append(np.concatenate([wcat, bcat], axis=0))  # [DM+1, 128]
            return _split_bf16(np.ascontiguousarray(np.stack(prs)))

        wqh, wql = pack_qk(W_Q, b_Q)
        wkh, wkl = pack_qk(W_K, b_K)
        wv_cat = np.concatenate([W_V[h] for h in heads], axis=1)
        bv_cat = np.concatenate([b_V[h] for h in heads])[None, :]
        wv_full = np.concatenate([wv_cat, bv_cat], axis=0).astype(ml_dtypes.bfloat16)
        in_maps.append(
            {
                "xh": np.ascontiguousarray(xh),
                "xl": np.ascontiguousarray(xl),
                "wqh": wqh,
                "wql": wql,
                "wkh": wkh,
                "wkl": wkl,
                "wv": np.ascontiguousarray(wv_full),
                "mask": mask,
                "ident": ident,
            }
        )
    return in_maps


def run_device(in_maps, bias_on=False, **kwargs):
    from concourse.bass_utils import run_bass_kernel_spmd

    key = ("nc", bias_on)
    if key not in _CACHE:
        _CACHE[key] = build_bass(bias_on)
    return run_bass_kernel_spmd(_CACHE[key], in_maps, core_ids=list(range(8)), **kwargs)


def kernel(normal_pre_resid, W_Q, W_K, W_V, W_O, b_Q, b_K, b_V, b_O, **extra):
    b_O = np.asarray(b_O, np.float32)
    bias_on = any(
        float(np.max(np.abs(np.asarray(bb, np.float32)))) > 0.0
        for bb in (b_Q, b_K, b_V)
    )
    in_maps = make_in_maps(
        normal_pre_resid, W_Q, W_K, W_V, W_O, b_Q, b_K, b_V, b_O
    )
    res = run_device(in_maps, bias_on=bias_on)
    W_O = np.asarray(W_O, np.float32)
    full = np.zeros((B, L, DM), np.float32)
    for c in range(8):
        b, hg = divmod(c, 4)
        heads = [4 * hg + j for j in range(HPC)]
        zo = np.asarray(res.results[c]["zout"], np.float32)  # [NG,P,NPAIR,G*P]
        # zo[g, h2*DH+hd, pr, s*P+qq] = unnormalized z for
        # q = g*G*P + s*P + qq, head = heads[pr*2+h2], dim hd
        zo = zo.reshape(NG, 2, DH, NPAIR, G, P)
        z = zo.transpose(0, 4, 5, 3, 1, 2).reshape(L, NPAIR * 2, DH)
        ss = np.asarray(res.results[c]["souts"], np.float64)  # [P,LT,NPAIR,2]
        ssq = ss.transpose(1, 0, 2, 3).reshape(L, NPAIR * 2)  # [q, pr*2+h2]
        z = z / ssq[:, :, None]
        for pi in range(NPAIR * 2):
            full[b] += z[:, pi, :] @ W_O[heads[pi]]
    full += b_O[None, None, :]
    return full
